# revision 40
# baseline (speedup 1.0000x reference)
"""Trainium2 Bass kernel for: softmax2d(channel) -> channel mix -> bias ->
RReLU(0.2 eval) -> relu(act + 0.1*x).

Full-input contract: kernel(**inputs) takes the complete tensors and returns
the complete output. Internally shards batch B=16 across 8 NeuronCores
(2 batches/core). Per-core layout: [128 partitions = 2 batches x 64 channels,
65536 free = H*W]. All HBM I/O in bf16 (in: 0.1*x pre-scaled on host; out: z),
halving DMA traffic vs f32 (DMA roofline ~360 B/ns/core).

Math restructure: with S[b,n] = sum_c e[(b,c),n] (e = exp(x), unnormalized)
and W'[(b,c),(b,d)] = mix[d,c] + bias[d] (bias folded into the weights),
  V' = W' @ e = S * (mix @ softmax + bias);   prelu(V')/S = prelu(V'/S)

Engine balance (HW rules: PSUM readable by ACT/DVE only, max ONE PSUM operand
per instruction, custom DVE ops always run 1x, TT/TS get 2x/4x on bf16 SBUF):
  e    = exp(10*x')        ACT  (x' = 0.1x bf16; scale=10)        -> bf16
  S    = blockones @ e     PE -> PSUM f32 (replicated over partitions)
  V'   = W' @ e            PE -> PSUM f32
  per chunk, main path (sigma ~ 55/64 of chunks):
    S_sb = copy(S)         ACT Copy PSUM -> SBUF bf16
    t    = prelu(V'*recip1nr(S_sb))   one fused custom DVE op (V' in PSUM)
  per chunk, alt path (rest; relieves ACT):
    rb   = recip1nr(S)     custom DVE (PSUM -> bf16)
    t    = prelu(V'*rb)    custom DVE (V' PSUM, rb SBUF)
  w = x' + t               DVE TT 2x-mode | Pool TT   (column split)
  z = max(w, 0)            DVE TS 4x-mode (in place)
  out = z                  DMA bf16; host upconverts to f32
"""

import numpy as np

B, C, H, W = 16, 64, 256, 256
N_CORES = 8
BPC = B // N_CORES          # batches per core
P = BPC * C                 # 128 partitions
F = H * W                   # 65536 free columns per core
TILE_N = 2048               # SBUF tile width
PS_N = 1024                 # PSUM chunk width (2 banks of f32)
MM_N = 512                  # single matmul max free dim (one PSUM bank, f32)
RRELU_SLOPE = 0.2
X_BUFS = 8                  # input prefetch depth
E_BUFS = 4
MID_BUFS = 4
W_BUFS = 4
PS_BUFS = 2
SKEWS = (0, 1, 2, 3, 4, 5)  # offsets: load, exp, mm, psum-consume, add, out
TAIL_DVE_FRAC = 0.5         # add+relu columns owned by DVE (rest Pool);
                            # disjoint ownership avoids cross-engine stalls
ALT_MOD = 7                 # chunks with (idx % ALT_MOD == 3) take alt path

_CACHE = {}

# Chebyshev-minimax seed constants for x*bitcast(~x) in [-4.5, -4]
# (see concourse.dve_ops.RECIPROCAL_APPROX_FAST); one Newton step -> ~0.4%.
RECIP_C0 = -0.23549792
RECIP_C1 = 2.0017324

FUSED_NAME = "PRELU_RECIPMUL_NN11888"
RECIP_NAME = "RECIP1NR_NN11888"
PRELUMUL_NAME = "PRELUMUL_NN11888"


def _register(name, spec_fn):
    import concourse.dve_ops as dve_ops
    from concourse.dve_spec import lower, _has_src1
    from concourse.dve_uop import DveOpSpec

    for op in dve_ops.OPS:
        if op.name == name:
            return op
    spec = spec_fn()
    op = dve_ops.DveOp(name, spec, subdim=False, uops_sha={})
    row = max(dve_ops._SUB_OPCODE_FOR_NAME.values()) + 1
    assert row < 0x20
    dve_ops.OPS.append(op)
    dve_ops._SUB_OPCODE_FOR_NAME[name] = row
    dve_ops.CUSTOM_DVE_SPECS[name] = spec
    for ver in ("v3", "v4"):
        dve_ops._COMPILE_CACHE[(name, ver)] = DveOpSpec(
            name=name,
            opcode=row,
            uops=lower(spec, ver=ver),
            rd1_en=_has_src1(spec),
        )
    return op


def _fused_op():
    """t = prelu_c2(Src0 * recip1nr(Src1)); Src0 = V' (PSUM f32),
    Src1 = S (SBUF bf16). 8 ALU ops -> 1 uop."""
    import numpy as np_
    from concourse.dve_spec import Spec, Src0, Src1, C0, C1, C2, maxx, Bin, AluOp

    def make():
        _not_s = Bin(AluOp.BITWISE_NOT, Src1, Src1)
        _y0 = _not_s * C0
        _y = _y0 * (C1 - Src1 * _y0)
        _q = Src0 * _y

        def ref(in0, in1, c0, c1, c2):
            s = in1.astype(np_.float32)
            not_s = (~s.view(np_.int32)).view(np_.float32)
            y0 = not_s * c0
            y = y0 * (c1 - s * y0)
            q = in0 * y
            return np_.maximum(q, q * c2)

        return Spec(body=maxx(_q, _q * C2), reference=ref)

    return _register(FUSED_NAME, make)


def _recip_op():
    """rb = recip1nr(Src0); Src0 = S (PSUM f32) -> bf16 SBUF. 5 ops."""
    import numpy as np_
    from concourse.dve_spec import Spec, Src0, C0, C1, Bin, AluOp

    def make():
        _not_s = Bin(AluOp.BITWISE_NOT, Src0, Src0)
        _y0 = _not_s * C0
        body = _y0 * (C1 - Src0 * _y0)

        def ref(in0, in1, c0, c1, c2):
            not_s = (~in0.view(np_.int32)).view(np_.float32)
            y0 = not_s * c0
            return y0 * (c1 - in0 * y0)

        return Spec(body=body, reference=ref)

    return _register(RECIP_NAME, make)


def _prelumul_op():
    """t = prelu_c0(Src0 * Src1); Src0 = V' (PSUM f32), Src1 = rb (SBUF bf16).
    3 ops."""
    import numpy as np_
    from concourse.dve_spec import Spec, Src0, Src1, C0, maxx

    def make():
        _q = Src0 * Src1

        def ref(in0, in1, c0, c1, c2):
            q = in0 * in1.astype(np_.float32)
            return np_.maximum(q, q * c0)

        return Spec(body=maxx(_q, _q * C0), reference=ref)

    return _register(PRELUMUL_NAME, make)


def _build_nc():
    import concourse.bacc as bacc
    import concourse.mybir as mybir
    import concourse.tile as tile

    f32 = mybir.dt.float32
    bf16 = mybir.dt.bfloat16
    AF = mybir.ActivationFunctionType
    OP = mybir.AluOpType

    nc = bacc.Bacc(
        "TRN2",
        target_bir_lowering=False,
        debug=False,
        enable_asserts=False,
    )

    x_d = nc.dram_tensor("x", [P, F], bf16, kind="ExternalInput").ap()
    wblk_d = nc.dram_tensor("wblk", [P, P], bf16, kind="ExternalInput").ap()
    ones_d = nc.dram_tensor("onesblk", [P, P], bf16, kind="ExternalInput").ap()
    out_d = nc.dram_tensor("out", [P, F], bf16, kind="ExternalOutput").ap()

    fused_op = _fused_op()
    recip_op = _recip_op()
    prelumul_op = _prelumul_op()

    tail_dve = int(TILE_N * TAIL_DVE_FRAC) // 128 * 128
    chunks_per_tile = TILE_N // PS_N

    with tile.TileContext(nc) as tc:
        with (
            tc.tile_pool(name="const", bufs=1) as const,
            tc.tile_pool(name="io", bufs=3) as io,
            tc.tile_pool(name="mid", bufs=MID_BUFS) as mid,
            tc.tile_pool(name="ps_s", bufs=1, space="PSUM") as ps_s,
            tc.tile_pool(name="ps_v", bufs=PS_BUFS, space="PSUM") as ps_v,
        ):
            w_mix = const.tile([P, P], bf16)
            nc.scalar.dma_start(out=w_mix[:], in_=wblk_d[:])
            onesblk = const.tile([P, P], bf16)
            nc.scalar.dma_start(out=onesblk[:], in_=ones_d[:])

            # warmup/cooldown mini-tiles shorten pipeline fill/drain
            NWARM = 2
            WARM_W = 1024
            tiles = (
                [(i * WARM_W, WARM_W) for i in range(NWARM)]
                + [(NWARM * WARM_W + i * TILE_N, TILE_N)
                   for i in range((F - 2 * NWARM * WARM_W) // TILE_N)]
                + [(F - (NWARM - i) * WARM_W, WARM_W) for i in range(NWARM)]
            )
            assert sum(w for _, w in tiles) == F
            ntiles = len(tiles)
            # alt tiles (DVE recip instead of ACT copy): balances ACT vs DVE
            ALT_TILES = {0, 8, 16, 24}
            st = {}  # per-tile live state

            def stage_load(ti):
                off, w = tiles[ti]
                x_t = io.tile(
                    [P, TILE_N], bf16, bufs=X_BUFS, name=f"x_{ti}", tag="x_t"
                )
                nc.sync.dma_start(out=x_t[:, :w], in_=x_d[:, off : off + w])
                st[ti] = {"x": x_t}

            def stage_exp(ti):
                w = tiles[ti][1]
                e_t = mid.tile(
                    [P, TILE_N], bf16, name=f"e_{ti}", tag="e_t", bufs=E_BUFS
                )
                nc.scalar.activation(
                    e_t[:, :w], st[ti]["x"][:, :w], AF.Exp, scale=10.0
                )
                st[ti]["e"] = e_t

            def stage_mm(ti):
                w = tiles[ti][1]
                e_t = st[ti]["e"]
                sb_t = ps_s.tile([P, TILE_N], f32, tag="sb_t")
                chunks = []
                for kp in range(0, w, PS_N):
                    cw = min(PS_N, w - kp)
                    v_c = ps_v.tile([P, PS_N], f32, tag="v_c")
                    for k in range(kp, kp + cw, MM_N):
                        nc.tensor.matmul(
                            sb_t[:, k : k + MM_N],
                            onesblk[:],
                            e_t[:, k : k + MM_N],
                            start=True,
                            stop=True,
                        )
                    for k in range(kp, kp + cw, MM_N):
                        nc.tensor.matmul(
                            v_c[:, k - kp : k - kp + MM_N],
                            w_mix[:],
                            e_t[:, k : k + MM_N],
                            start=True,
                            stop=True,
                        )
                    chunks.append((kp, cw, v_c))
                st[ti]["sb"] = sb_t
                st[ti]["chunks"] = chunks

            def stage_scopy(ti):
                # move S out of PSUM: ACT copy (main) or DVE recip (alt),
                # whole tile in one instruction
                w = tiles[ti][1]
                ssb_t = mid.tile([P, TILE_N], bf16, name=f"ss_{ti}",
                                 tag="ssb_t")
                sb_t = st[ti].pop("sb")
                alt = ti in ALT_TILES
                if alt:
                    nc.vector._custom_dve(
                        recip_op,
                        out=ssb_t[:, :w],
                        in0=sb_t[:, :w],
                        s0=RECIP_C0,
                        s1=RECIP_C1,
                    )
                else:
                    nc.scalar.copy(ssb_t[:, :w], sb_t[:, :w])
                st[ti]["ssb"] = ssb_t
                st[ti]["alt"] = alt

            def stage_fused(ti):
                # consume V' PSUM chunks -> t (prelu'd, normalized) bf16
                t_t = mid.tile([P, TILE_N], bf16, name=f"t_{ti}", tag="t_t")
                ssb_t = st[ti].pop("ssb")
                alt = st[ti].pop("alt")
                for kp, cw, v_c in st[ti].pop("chunks"):
                    psl = slice(kp, kp + cw)
                    if alt:
                        nc.vector._custom_dve(
                            prelumul_op,
                            out=t_t[:, psl],
                            in0=v_c[:, :cw],
                            in1=ssb_t[:, psl],
                            s0=RRELU_SLOPE,
                        )
                    else:
                        nc.vector._custom_dve(
                            fused_op,
                            out=t_t[:, psl],
                            in0=v_c[:, :cw],
                            in1=ssb_t[:, psl],
                            s0=RECIP_C0,
                            s1=RECIP_C1,
                            imm2=RRELU_SLOPE,
                        )
                st[ti]["t"] = t_t

            def stage_add(ti):
                w = tiles[ti][1]
                frac = TAIL_DVE_FRAC
                if ti >= ntiles - 3:
                    frac = 0.8
                elif ti < 3:
                    frac = 0.25
                td = int(w * frac) // 128 * 128
                x_t, t_t = st[ti]["x"], st[ti]["t"]
                w_t = mid.tile([P, TILE_N], bf16, name=f"w_{ti}", tag="w_t")
                if td > 0:
                    nc.vector.tensor_tensor(
                        w_t[:, :td], x_t[:, :td], t_t[:, :td], OP.add,
                    )
                if td < w:
                    nc.gpsimd.tensor_tensor(
                        w_t[:, td:w], x_t[:, td:w], t_t[:, td:w], OP.add,
                    )
                st[ti]["w"] = w_t
                st[ti]["td"] = td

            def stage_out(ti):
                off, w = tiles[ti]
                td = st[ti]["td"]
                w_t = st[ti]["w"]
                z_t = io.tile([P, TILE_N], bf16, bufs=W_BUFS,
                              name=f"z_{ti}", tag="z_t")
                # z = max(w, 0); each engine owns the columns it added
                if td > 0:
                    nc.vector.tensor_scalar(
                        out=z_t[:, :td], in0=w_t[:, :td],
                        scalar1=0.0, scalar2=None, op0=OP.max,
                    )
                if td < w:
                    nc.gpsimd.tensor_scalar(
                        out=z_t[:, td:w], in0=w_t[:, td:w],
                        scalar1=0.0, scalar2=None, op0=OP.max,
                    )
                nc.sync.dma_start(
                    out=out_d[:, off : off + w],
                    in_=z_t[:, :w],
                )
                del st[ti]

            stages = [stage_load, stage_exp, stage_mm, stage_scopy,
                      stage_fused, stage_add, stage_out]
            offs = (0, 1, 2, 3, 4, 5, 6)
            assert len(offs) == len(stages)
            maxoff = offs[-1]
            # emission order per step: load/exp first (keeps ACT's exp ahead
            # of its copies in queue), then deepest stages first
            emit_order = [0, 1, 6, 5, 4, 3, 2]
            for step in range(ntiles + maxoff):
                for si in emit_order:
                    ti = step - offs[si]
                    if 0 <= ti < ntiles:
                        stages[si](ti)

    nc.compile()
    return nc


def _get_nc():
    if "nc" not in _CACHE:
        _CACHE["nc"] = _build_nc()
    return _CACHE["nc"]


def _make_in_maps(x, mix, bias):
    import ml_dtypes

    bf = ml_dtypes.bfloat16
    x = np.asarray(x, dtype=np.float32)
    mix = np.asarray(mix, dtype=np.float32)
    bias = np.asarray(bias, dtype=np.float32)

    # pre-scale the residual: x' = 0.1x (exp compensates with scale=10)
    xs = np.ascontiguousarray((x * np.float32(0.1)).astype(bf).reshape(
        N_CORES, P, F))

    # lhsT layout: V'[(b,d),n] = sum_{(b',c)} wblk[(b',c),(b,d)] * e[(b',c),n]
    # wblk[(b,c),(b,d)] = mix[d,c] + bias[d]  (bias folded: sums to bias*S)
    blk = (mix.T + bias[None, :]).astype(bf)
    wblk = np.zeros((P, P), bf)
    wblk[0:C, 0:C] = blk
    wblk[C : 2 * C, C : 2 * C] = blk

    onesblk = np.zeros((P, P), bf)
    onesblk[0:C, 0:C] = 1.0
    onesblk[C : 2 * C, C : 2 * C] = 1.0

    return [
        {"x": xs[c], "wblk": wblk, "onesblk": onesblk}
        for c in range(N_CORES)
    ]


def run(inputs, trace=False):
    from concourse.bass_utils import run_bass_kernel_spmd

    nc = _get_nc()
    in_maps = _make_in_maps(inputs["x"], inputs["mix"], inputs["bias"])
    res = run_bass_kernel_spmd(nc, in_maps, list(range(N_CORES)), trace=trace)
    out = np.stack(
        [np.asarray(res.results[c]["out"]) for c in range(N_CORES)]
    ).astype(np.float32)
    return out.reshape(B, C, H, W), res


def kernel(x, mix, bias):
    out, _ = run({"x": x, "mix": mix, "bias": bias})
    return out.astype(np.float32, copy=False)
